# revision 14
# baseline (speedup 1.0000x reference)
"""Distributed Trainium2 Bass kernel for the SupCon-style partial-label
contrastive loss (nn_ABLE_48198122995803).

Sharding: the 4096 anchor rows of the logits matrix are split across the 8
NeuronCores (512 rows each).  contrast_feature^T is replicated to every core
(in fp16); each core computes its row-block of logits, the masked row
reductions, and a partial loss sum.  The pseudo-label argmax (tp) is sharded
8x256 rows and exchanged with one tiny fp16 AllGather.  new_target is
row-sharded 8x256.

Math notes (vs reference.py):
 - features are L2-normalized, so row-max(adc) == adc_ii ~= 10.  log_prob is
   invariant to the per-row subtracted constant, so we use the constant 10;
   exp(l_ii) ~= 1 (error < 2e-5) is excluded from the denominator as 1.0.
 - mask[a,b] = W[a, tp[b]] is the one-hot matmul W^T.T @ S with
   S[c,b] = (tp[b]==c) in fp16 (exact gather; W rounded to fp16, err 5e-4).
 - per-row: row_loss = 10*s1 - 20*Mh - lnZ*(2*Mh - md), where
   s1 = sum_b mask[a,b]*qraw[i,b], qraw = dot[i,b]+dot[i,b+2048],
   Mh = sum_b mask[a,b] (counts matmul), md = mask[a,a].
   loss = -(10/7) * mean(row_loss).

Layout: adc PSUM tiles are [128, 2048] (4 banks), two per row-tile m in
chunk-pair order (n0, n0+4, n1, n1+4) so the qraw pair-adds read one tile;
one big ScalarE exp+accum per tile gives the Z row-sums.
"""

import numpy as np

import concourse.bass as bass
import concourse.tile as tile
from concourse import bacc, mybir
from concourse.bass_utils import run_bass_kernel_spmd

B = 2048
C = 1000
CP = 1024          # class dim padded to 8*128
D = 256
TWOB = 4096
NCORES = 8
RPC = TWOB // NCORES   # 512 logits rows per core
BSH = B // NCORES      # 256 softmax/new_target rows per core

F32 = mybir.dt.float32
F16 = mybir.dt.float16
U16 = mybir.dt.uint16
AX = mybir.AxisListType
OP = mybir.AluOpType
ACT = mybir.ActivationFunctionType


def build_graph():
    nc = bacc.Bacc("TRN2", target_bir_lowering=False, debug=False,
                   num_devices=NCORES)

    # ---- per-core external I/O ------------------------------------------
    cfl = nc.dram_tensor("cfl", [D, RPC], F16, kind="ExternalInput")
    cft = nc.dram_tensor("cft", [D, TWOB], F16, kind="ExternalInput")
    osh = nc.dram_tensor("osh", [BSH, C], F32, kind="ExternalInput")
    ysh = nc.dram_tensor("ysh", [BSH, C], F32, kind="ExternalInput")
    ytb = nc.dram_tensor("ytb", [CP, RPC], F32, kind="ExternalInput")
    psb = nc.dram_tensor("psb", [CP, RPC], F32, kind="ExternalInput")
    dsel = nc.dram_tensor("dsel", [128, 4], F32, kind="ExternalInput")
    out_nt = nc.dram_tensor("out_nt", [BSH, C], F32, kind="ExternalOutput")
    out_loss = nc.dram_tensor("out_loss", [128, 1], F32, kind="ExternalOutput")

    # ---- inline constants ----------------------------------------------
    eye_np = np.eye(128, dtype=np.float32)
    io_np = (np.arange(128, dtype=np.float32)[:, None]
             + 128.0 * np.arange(8, dtype=np.float32)[None, :])
    eye_dr = nc.inline_tensor(eye_np, name="eye_c")
    io_dr = nc.inline_tensor(io_np, name="io_c")
    one_dr = nc.inline_tensor(np.ones((1, 128), dtype=np.float16), name="one_c")

    # ---- collective bounce buffers -------------------------------------
    cc_in = nc.dram_tensor("cc_in", [BSH], F16, kind="Internal",
                           addr_space="Local")
    cc_out = nc.dram_tensor("cc_out", [B], F16, kind="Internal",
                            addr_space="Shared")

    with tile.TileContext(nc) as tc:
        with (
            tc.tile_pool(name="persist", bufs=1) as pp,
            tc.tile_pool(name="scr", bufs=2) as sp,
            tc.tile_pool(name="psum", bufs=2, space="PSUM") as psp,
        ):
            # ---------- constants into SBUF ----------
            eye_sb = pp.tile([128, 128], F32, tag="eye")
            io_sb = pp.tile([128, 8], F32, tag="io")
            one_sb = pp.tile([1, 128], F16, tag="one")
            dsel_sb = pp.tile([128, 4], F32, tag="dsel")
            nb10 = pp.tile([128, 1], F32, tag="nb10")
            nc.vector.memset(nb10[:], -10.0)
            nc.sync.dma_start(out=eye_sb[:], in_=eye_dr.ap())
            nc.sync.dma_start(out=io_sb[:], in_=io_dr.ap())
            nc.sync.dma_start(out=one_sb[:], in_=one_dr.ap())
            nc.sync.dma_start(out=dsel_sb[:], in_=dsel.ap())

            # ---------- shard inputs for tp + new_target ----------
            osh_t, ysh_t = [], []
            for j in range(2):
                ot = pp.tile([128, C], F32, tag=f"osh{j}")
                yt = pp.tile([128, C], F32, tag=f"ysh{j}")
                nc.sync.dma_start(out=ot[:], in_=osh.ap()[j * 128:(j + 1) * 128, :])
                nc.sync.dma_start(out=yt[:], in_=ysh.ap()[j * 128:(j + 1) * 128, :])
                osh_t.append(ot)
                ysh_t.append(yt)

            # ---------- phase 1: tp shard (argmax over candidates) ----------
            tp16 = pp.tile([128, 2], F16, tag="tp16")
            for j in range(2):
                v = sp.tile([128, C], F32, tag="vmask")
                # v = (o + 1000) * Y : >0 on candidates, 0 elsewhere
                nc.vector.scalar_tensor_tensor(
                    out=v[:], in0=osh_t[j][:], scalar=1000.0, in1=ysh_t[j][:],
                    op0=OP.add, op1=OP.mult)
                mx8 = sp.tile([128, 8], F32, tag="mx8")
                idx8 = sp.tile([128, 8], U16, tag="idx8")
                nc.vector.max(out=mx8[:], in_=v[:])
                nc.vector.max_index(out=idx8[:], in_max=mx8[:], in_values=v[:])
                idf = sp.tile([128, 1], F32, tag="idf")
                nc.vector.tensor_copy(out=idf[:], in_=idx8[:, 0:1])
                nc.vector.tensor_copy(out=tp16[:, j:j + 1], in_=idf[:])

            # ---------- phase 2: AllGather tp ----------
            for j in range(2):
                nc.gpsimd.dma_start(out=cc_in.ap()[j * 128:(j + 1) * 128],
                                    in_=tp16[:, j:j + 1])
            nc.gpsimd.collective_compute(
                "AllGather", OP.bypass,
                replica_groups=[list(range(NCORES))],
                ins=[cc_in.ap().opt()],
                outs=[cc_out.ap().opt()],
            )
            tpb = pp.tile([128, B], F16, tag="tpb")
            cc_bcast = bass.AP(tensor=cc_out.ap().tensor, offset=0,
                               ap=[[0, 128], [1, B]])
            nc.gpsimd.dma_start(out=tpb[:], in_=cc_bcast)

            # ---------- phase 3: main matmul inputs ----------
            cfl_t, cft_t = [], []
            for k in range(2):
                lt = pp.tile([128, RPC], F16, tag=f"cfl{k}")
                ft = pp.tile([128, TWOB], F16, tag=f"cft{k}")
                nc.sync.dma_start(out=lt[:], in_=cfl.ap()[k * 128:(k + 1) * 128, :])
                nc.sync.dma_start(out=ft[:], in_=cft.ap()[k * 128:(k + 1) * 128, :])
                cfl_t.append(lt)
                cft_t.append(ft)
            ytb_t, psb_t = [], []
            for t in range(8):
                yt = pp.tile([128, RPC], F32, tag=f"ytb{t}")
                pt = pp.tile([128, RPC], F32, tag=f"psb{t}")
                nc.sync.dma_start(out=yt[:], in_=ytb.ap()[t * 128:(t + 1) * 128, :])
                nc.sync.dma_start(out=pt[:], in_=psb.ap()[t * 128:(t + 1) * 128, :])
                ytb_t.append(yt)
                psb_t.append(pt)

            # ---------- phase 4: adc matmul, exp/Z, qraw ----------
            # psum tile per (m, half): columns = chunks (n0, n0+4, n1, n1+4),
            # n0 = 2h, n1 = 2h+1.
            zacc = pp.tile([128, 4, 2], F32, tag="zacc")
            qraw = [pp.tile([128, B], F16, tag=f"qraw{m}", name=f"qraw{m}")
                    for m in range(4)]
            for m in range(4):
                for h in range(2):
                    pst = psp.tile([128, 2048], F32, tag="ps",
                                   name=f"psadc{m}_{h}")
                    chunk_cols = [2 * h, 2 * h + 4, 2 * h + 1, 2 * h + 5]
                    for k in range(2):
                        lhsT = cfl_t[k][:, m * 128:(m + 1) * 128]
                        for c, n in enumerate(chunk_cols):
                            nc.tensor.matmul(
                                pst[:, c * 512:(c + 1) * 512],
                                lhsT,
                                cft_t[k][:, n * 512:(n + 1) * 512],
                                start=(k == 0), stop=(k == 1))
                    # Z partial: one exp+accum over the whole 4-bank tile
                    esc = sp.tile([128, 2048], F32, tag="escr")
                    nc.scalar.activation(
                        out=esc[:], in_=pst[:], func=ACT.Exp,
                        bias=nb10[:], scale=10.0,
                        accum_out=zacc[:, m, h:h + 1])
                    # qraw pair sums: pst as [128, 2 pairs, 2 half, 512]
                    pst4 = pst[:].rearrange("p (a b c) -> p a b c", a=2, b=2)
                    ucp = sp.tile([128, 2, 512], F32, tag="ucp",
                                  name=f"ucp{m}_{h}")
                    nc.scalar.activation(out=ucp[:], in_=pst4[:, :, 1, :],
                                         func=ACT.Copy)
                    nc.vector.tensor_add(
                        qraw[m][:, h * 1024:(h + 1) * 1024]
                        .rearrange("p (a c) -> p a c", a=2),
                        pst4[:, :, 0, :], ucp[:])

            # ---------- phase 5: build S + counts ----------
            counts = pp.tile([128, 8], F32, tag="counts")
            s_t = [pp.tile([128, B], F16, tag=f"s{t}", name=f"s{t}")
                   for t in range(8)]
            for t in range(8):
                nc.gpsimd.tensor_scalar(
                    out=s_t[t][:], in0=tpb[:], scalar1=io_sb[:, t:t + 1],
                    scalar2=None, op0=OP.is_equal)
                nc.vector.tensor_reduce(
                    counts[:, t:t + 1], s_t[t][:], axis=AX.X, op=OP.add)

            cm = pp.tile([128, 8], F32, tag="cm")
            cinv = pp.tile([128, 8], F32, tag="cinv")
            c16 = pp.tile([128, 8], F16, tag="c16")
            nc.vector.tensor_scalar_max(cm[:], counts[:], 1.0)
            nc.vector.reciprocal(out=cinv[:], in_=cm[:])
            nc.scalar.activation(out=c16[:], in_=counts[:], func=ACT.Copy)

            # ---------- phase 6: W^T in fp16 ----------
            wt16 = [pp.tile([128, RPC], F16, tag=f"wt{t}", name=f"wt{t}")
                    for t in range(8)]
            for t in range(8):
                nc.gpsimd.tensor_mul(ytb_t[t][:], ytb_t[t][:], psb_t[t][:])
                nc.scalar.activation(out=wt16[t][:], in_=ytb_t[t][:],
                                     func=ACT.Copy, scale=cinv[:, t:t + 1])

            # ---------- phase 8: mask matmul + masked row sums ----------
            s1 = pp.tile([128, 4], F32, tag="s1a")
            mdc = pp.tile([128, 4, 4], F32, tag="mdc")
            for m in range(4):
                psk = psp.tile([128, 2048], F32, tag="ps", name=f"psmask{m}")
                for t in range(8):
                    lhsT = wt16[t][:, m * 128:(m + 1) * 128]
                    for n in range(4):
                        nc.tensor.matmul(
                            psk[:, n * 512:(n + 1) * 512], lhsT,
                            s_t[t][:, n * 512:(n + 1) * 512],
                            start=(t == 0), stop=(t == 7))
                tsc = sp.tile([128, 2048], F32, tag="tscr")
                nc.vector.scalar_tensor_tensor(
                    out=tsc[:], in0=psk[:], scalar=1.0, in1=qraw[m][:],
                    op0=OP.mult, op1=OP.mult,
                    accum_out=s1[:, m:m + 1])
                for n in range(4):
                    t128 = sp.tile([128, 128], F32, tag="t128")
                    nc.vector.scalar_tensor_tensor(
                        out=t128[:],
                        in0=psk[:, n * 512 + m * 128:n * 512 + (m + 1) * 128],
                        scalar=1.0, in1=eye_sb[:],
                        op0=OP.mult, op1=OP.mult,
                        accum_out=mdc[:, m, n:n + 1])

            # ---------- phase 7: Mh via counts matmul ----------
            ps_m = psp.tile([128, 4], F32, tag="ps", name="ps_m")
            mh = pp.tile([128, 4], F32, tag="mh")
            for m in range(4):
                for t in range(8):
                    nc.tensor.matmul(
                        ps_m[:, m:m + 1],
                        wt16[t][:, m * 128:(m + 1) * 128],
                        c16[:, t:t + 1],
                        start=(t == 0), stop=(t == 7))
            nc.vector.tensor_copy(out=mh[:], in_=ps_m[:])

            # ---------- phase 9: per-row loss ----------
            zsum = pp.tile([128, 4], F32, tag="zsum")
            zz = pp.tile([128, 4], F32, tag="zz")
            lnz = pp.tile([128, 4], F32, tag="lnz")
            nc.vector.reduce_sum(zsum[:], zacc[:], axis=AX.X)
            nc.vector.tensor_scalar_add(zz[:], zsum[:], -1.0)
            nc.scalar.activation(out=lnz[:], in_=zz[:], func=ACT.Ln)

            md = pp.tile([128, 4], F32, tag="md")
            for m in range(4):
                t4 = sp.tile([128, 4], F32, tag="t4")
                nc.vector.scalar_tensor_tensor(
                    out=t4[:], in0=mdc[:, m, :], scalar=1.0, in1=dsel_sb[:],
                    op0=OP.mult, op1=OP.mult,
                    accum_out=md[:, m:m + 1])

            t1 = pp.tile([128, 4], F32, tag="t1")
            t2 = pp.tile([128, 4], F32, tag="t2")
            u1 = pp.tile([128, 4], F32, tag="u1")
            rl = pp.tile([128, 4], F32, tag="rl")
            rlp = pp.tile([128, 1], F32, tag="rlp")
            # t1 = 2*Mh - md ; t2 = lnz*t1 ; u1 = 10*s1 - t2 ; rl = -20*Mh + u1
            nc.vector.scalar_tensor_tensor(out=t1[:], in0=mh[:], scalar=2.0,
                                           in1=md[:], op0=OP.mult,
                                           op1=OP.subtract)
            nc.vector.tensor_mul(t2[:], lnz[:], t1[:])
            nc.vector.scalar_tensor_tensor(out=u1[:], in0=s1[:], scalar=10.0,
                                           in1=t2[:], op0=OP.mult,
                                           op1=OP.subtract)
            nc.vector.scalar_tensor_tensor(out=rl[:], in0=mh[:], scalar=-20.0,
                                           in1=u1[:], op0=OP.mult, op1=OP.add)
            nc.vector.reduce_sum(rlp[:], rl[:], axis=AX.X)
            nc.sync.dma_start(out=out_loss.ap(), in_=rlp[:])

            # ---------- phase 10: new_target ----------
            snt = pp.tile([128, 2], F32, tag="snt")
            rin = pp.tile([128, 2], F32, tag="rin")
            for j in range(2):
                mx = sp.tile([128, 1], F32, tag="ntmx")
                nmx = sp.tile([128, 1], F32, tag="ntnmx")
                nc.vector.reduce_max(mx[:], osh_t[j][:], axis=AX.X)
                nc.scalar.activation(out=nmx[:], in_=mx[:], func=ACT.Copy,
                                     scale=-1.0)
                ee = sp.tile([128, C], F32, tag="ntee")
                nc.scalar.activation(out=ee[:], in_=osh_t[j][:], func=ACT.Exp,
                                     bias=nmx[:], scale=1.0)
                tt = sp.tile([128, C], F32, tag="nttt")
                nc.vector.scalar_tensor_tensor(
                    out=tt[:], in0=ee[:], scalar=1.0, in1=ysh_t[j][:],
                    op0=OP.mult, op1=OP.mult,
                    accum_out=snt[:, j:j + 1])
                nc.vector.reciprocal(out=rin[:, j:j + 1], in_=snt[:, j:j + 1])
                nc.scalar.activation(out=tt[:], in_=tt[:], func=ACT.Copy,
                                     scale=rin[:, j:j + 1])
                nc.sync.dma_start(out=out_nt.ap()[j * 128:(j + 1) * 128, :],
                                  in_=tt[:])

    nc.compile()
    return nc


_NC_CACHE = None


def _get_graph():
    global _NC_CACHE
    if _NC_CACHE is None:
        _NC_CACHE = build_graph()
    return _NC_CACHE


def make_in_maps(outputs, features, Y, predicted_score):
    outputs = np.ascontiguousarray(outputs, dtype=np.float32)
    features = np.ascontiguousarray(features, dtype=np.float32)
    Y = np.ascontiguousarray(Y, dtype=np.float32)
    ps = np.ascontiguousarray(predicted_score, dtype=np.float32)

    cf = np.concatenate([features[:, 0, :], features[:, 1, :]], axis=0)
    cft = np.ascontiguousarray(cf.T.astype(np.float16))     # [256, 4096]
    in_maps = []
    for r in range(NCORES):
        ablk = 512 * (r % 4)
        ytb = np.zeros((CP, RPC), dtype=np.float32)
        psb = np.zeros((CP, RPC), dtype=np.float32)
        ytb[:C] = Y[ablk:ablk + RPC].T
        psb[:C] = ps[ablk:ablk + RPC].T
        dsel = np.zeros((128, 4), dtype=np.float32)
        dsel[:, r % 4] = 1.0
        in_maps.append({
            "cfl": np.ascontiguousarray(cft[:, r * RPC:(r + 1) * RPC]),
            "cft": cft,
            "osh": np.ascontiguousarray(outputs[r * BSH:(r + 1) * BSH]),
            "ysh": np.ascontiguousarray(Y[r * BSH:(r + 1) * BSH]),
            "ytb": ytb,
            "psb": psb,
            "dsel": dsel,
        })
    return in_maps


def assemble(results):
    new_target = np.concatenate(
        [results[r]["out_nt"] for r in range(NCORES)], axis=0)
    total = np.float64(0.0)
    for r in range(NCORES):
        total += np.float64(results[r]["out_loss"].sum())
    loss = -(10.0 / 7.0) * total / float(TWOB)
    return np.float32(loss), new_target.astype(np.float32)


def kernel(outputs, features, Y, predicted_score):
    nc = _get_graph()
    in_maps = make_in_maps(outputs, features, Y, predicted_score)
    res = run_bass_kernel_spmd(nc, in_maps, core_ids=list(range(NCORES)))
    return assemble(res.results)


# revision 15
# speedup vs baseline: 2.1805x; 2.1805x over previous
"""Distributed Trainium2 Bass kernel for the SupCon-style partial-label
contrastive loss (nn_ABLE_48198122995803).

Sharding: the 4096 anchor rows of the logits matrix are split across the 8
NeuronCores (512 rows each).  contrast_feature^T is replicated to every core
(in fp16); each core computes its row-block of logits, the masked row
reductions, and a partial loss sum.  The pseudo-label argmax (tp) is sharded
8x256 rows and exchanged with one tiny fp16 AllGather.  new_target is
row-sharded 8x256.

Math notes (vs reference.py):
 - features are L2-normalized, so row-max(adc) == adc_ii ~= 10.  log_prob is
   invariant to the per-row subtracted constant, so we use the constant 10;
   exp(l_ii) ~= 1 (error < 2e-5) is excluded from the denominator as 1.0.
 - mask[a,b] = W[a, tp[b]] is the one-hot matmul W^T.T @ S with
   S[c,b] = (tp[b]==c) in fp16 (exact gather; W rounded to fp16, err 5e-4).
 - per-row: row_loss = 10*s1 - 20*Mh - lnZ*(2*Mh - md), where
   s1 = sum_b mask[a,b]*qraw[i,b], qraw = dot[i,b]+dot[i,b+2048],
   Mh = sum_b mask[a,b] (counts matmul), md = mask[a,a].
   loss = -(10/7) * mean(row_loss).

Layout: adc PSUM tiles are [128, 2048] (4 banks), two per row-tile m in
chunk-pair order (n0, n0+4, n1, n1+4) so the qraw pair-adds read one tile;
one big ScalarE exp+accum per tile gives the Z row-sums.
"""

import numpy as np

import concourse.bass as bass
import concourse.tile as tile
from concourse import bacc, mybir
from concourse.bass_utils import run_bass_kernel_spmd

B = 2048
C = 1000
CP = 1024          # class dim padded to 8*128
D = 256
TWOB = 4096
NCORES = 8
RPC = TWOB // NCORES   # 512 logits rows per core
BSH = B // NCORES      # 256 softmax/new_target rows per core

F32 = mybir.dt.float32
F16 = mybir.dt.float16
U16 = mybir.dt.uint16
AX = mybir.AxisListType
OP = mybir.AluOpType
ACT = mybir.ActivationFunctionType


def build_graph():
    nc = bacc.Bacc("TRN2", target_bir_lowering=False, debug=False,
                   num_devices=NCORES)

    # ---- per-core external I/O ------------------------------------------
    cfl = nc.dram_tensor("cfl", [D, RPC], F16, kind="ExternalInput")
    cft = nc.dram_tensor("cft", [D, TWOB], F16, kind="ExternalInput")
    osh = nc.dram_tensor("osh", [BSH, C], F32, kind="ExternalInput")
    ysh = nc.dram_tensor("ysh", [BSH, C], F32, kind="ExternalInput")
    ytb = nc.dram_tensor("ytb", [CP, RPC], F32, kind="ExternalInput")
    psb = nc.dram_tensor("psb", [CP, RPC], F32, kind="ExternalInput")
    dsel = nc.dram_tensor("dsel", [128, 4], F32, kind="ExternalInput")
    out_nt = nc.dram_tensor("out_nt", [BSH, C], F32, kind="ExternalOutput")
    out_loss = nc.dram_tensor("out_loss", [128, 1], F32, kind="ExternalOutput")

    # ---- inline constants ----------------------------------------------
    eye_np = np.eye(128, dtype=np.float32)
    io_np = (np.arange(128, dtype=np.float32)[:, None]
             + 128.0 * np.arange(8, dtype=np.float32)[None, :])
    eye_dr = nc.inline_tensor(eye_np, name="eye_c")
    io_dr = nc.inline_tensor(io_np, name="io_c")
    one_dr = nc.inline_tensor(np.ones((1, 128), dtype=np.float16), name="one_c")

    # ---- collective bounce buffers -------------------------------------
    cc_in = nc.dram_tensor("cc_in", [BSH], F16, kind="Internal",
                           addr_space="Local")
    cc_out = nc.dram_tensor("cc_out", [B], F16, kind="Internal",
                            addr_space="Shared")

    with tile.TileContext(nc) as tc:
        with (
            tc.tile_pool(name="persist", bufs=1) as pp,
            tc.tile_pool(name="scr", bufs=2) as sp,
            tc.tile_pool(name="psum", bufs=2, space="PSUM") as psp,
        ):
            # ---------- constants into SBUF ----------
            eye_sb = pp.tile([128, 128], F32, tag="eye")
            io_sb = pp.tile([128, 8], F32, tag="io")
            one_sb = pp.tile([1, 128], F16, tag="one")
            dsel_sb = pp.tile([128, 4], F32, tag="dsel")
            nb10 = pp.tile([128, 1], F32, tag="nb10")
            nc.vector.memset(nb10[:], -10.0)
            nc.sync.dma_start(out=eye_sb[:], in_=eye_dr.ap())
            nc.sync.dma_start(out=io_sb[:], in_=io_dr.ap())
            nc.sync.dma_start(out=one_sb[:], in_=one_dr.ap())
            nc.sync.dma_start(out=dsel_sb[:], in_=dsel.ap())

            # ---------- shard inputs for tp + new_target ----------
            osh_t, ysh_t = [], []
            for j in range(2):
                ot = pp.tile([128, C], F32, tag=f"osh{j}")
                yt = pp.tile([128, C], F32, tag=f"ysh{j}")
                nc.sync.dma_start(out=ot[:], in_=osh.ap()[j * 128:(j + 1) * 128, :])
                nc.sync.dma_start(out=yt[:], in_=ysh.ap()[j * 128:(j + 1) * 128, :])
                osh_t.append(ot)
                ysh_t.append(yt)

            # ---------- phase 1: tp shard (argmax over candidates) ----------
            tp16 = pp.tile([128, 2], F16, tag="tp16")
            for j in range(2):
                v = sp.tile([128, C], F32, tag="vmask")
                # v = (o + 1000) * Y : >0 on candidates, 0 elsewhere
                nc.vector.scalar_tensor_tensor(
                    out=v[:], in0=osh_t[j][:], scalar=1000.0, in1=ysh_t[j][:],
                    op0=OP.add, op1=OP.mult)
                mx8 = sp.tile([128, 8], F32, tag="mx8")
                idx8 = sp.tile([128, 8], U16, tag="idx8")
                nc.vector.max(out=mx8[:], in_=v[:])
                nc.vector.max_index(out=idx8[:], in_max=mx8[:], in_values=v[:])
                idf = sp.tile([128, 1], F32, tag="idf")
                nc.vector.tensor_copy(out=idf[:], in_=idx8[:, 0:1])
                nc.vector.tensor_copy(out=tp16[:, j:j + 1], in_=idf[:])

            # ---------- phase 2: AllGather tp ----------
            for j in range(2):
                nc.gpsimd.dma_start(out=cc_in.ap()[j * 128:(j + 1) * 128],
                                    in_=tp16[:, j:j + 1])
            nc.gpsimd.collective_compute(
                "AllGather", OP.bypass,
                replica_groups=[list(range(NCORES))],
                ins=[cc_in.ap().opt()],
                outs=[cc_out.ap().opt()],
            )
            tpb = pp.tile([128, B], F16, tag="tpb")
            cc_bcast = bass.AP(tensor=cc_out.ap().tensor, offset=0,
                               ap=[[0, 128], [1, B]])
            nc.gpsimd.dma_start(out=tpb[:], in_=cc_bcast)

            # ---------- phase 3: main matmul inputs ----------
            cfl_t, cft_t = [], []
            for k in range(2):
                lt = pp.tile([128, RPC], F16, tag=f"cfl{k}")
                ft = pp.tile([128, TWOB], F16, tag=f"cft{k}")
                nc.sync.dma_start(out=lt[:], in_=cfl.ap()[k * 128:(k + 1) * 128, :])
                nc.sync.dma_start(out=ft[:], in_=cft.ap()[k * 128:(k + 1) * 128, :])
                cfl_t.append(lt)
                cft_t.append(ft)
            ytb_t, psb_t = [], []
            for t in range(8):
                yt = pp.tile([128, RPC], F32, tag=f"ytb{t}")
                pt = pp.tile([128, RPC], F32, tag=f"psb{t}")
                nc.sync.dma_start(out=yt[:], in_=ytb.ap()[t * 128:(t + 1) * 128, :])
                nc.sync.dma_start(out=pt[:], in_=psb.ap()[t * 128:(t + 1) * 128, :])
                ytb_t.append(yt)
                psb_t.append(pt)

            # ---------- phase 4: adc matmul, exp/Z, qraw ----------
            # psum tile per (m, half): columns = chunks (n0, n0+4, n1, n1+4),
            # n0 = 2h, n1 = 2h+1.
            zacc = pp.tile([128, 4, 2], F32, tag="zacc")
            qraw = [pp.tile([128, B], F16, tag=f"qraw{m}", name=f"qraw{m}")
                    for m in range(4)]
            for m in range(4):
                for h in range(2):
                    pst = psp.tile([128, 2048], F32, tag="ps",
                                   name=f"psadc{m}_{h}")
                    chunk_cols = [2 * h, 2 * h + 4, 2 * h + 1, 2 * h + 5]
                    for k in range(2):
                        lhsT = cfl_t[k][:, m * 128:(m + 1) * 128]
                        for c, n in enumerate(chunk_cols):
                            nc.tensor.matmul(
                                pst[:, c * 512:(c + 1) * 512],
                                lhsT,
                                cft_t[k][:, n * 512:(n + 1) * 512],
                                start=(k == 0), stop=(k == 1))
                    # Z partial: one exp+accum over the whole 4-bank tile
                    esc = sp.tile([128, 2048], F32, tag="escr")
                    nc.scalar.activation(
                        out=esc[:], in_=pst[:], func=ACT.Exp,
                        bias=nb10[:], scale=10.0,
                        accum_out=zacc[:, m, h:h + 1])
                    # qraw pair sums: pst as [128, 2 pairs, 2 half, 512]
                    pst4 = pst[:].rearrange("p (a b c) -> p a b c", a=2, b=2)
                    ucp = sp.tile([128, 2, 512], F32, tag="ucp",
                                  name=f"ucp{m}_{h}")
                    nc.scalar.activation(out=ucp[:], in_=pst4[:, :, 1, :],
                                         func=ACT.Copy)
                    nc.vector.tensor_add(
                        qraw[m][:, h * 1024:(h + 1) * 1024]
                        .rearrange("p (a c) -> p a c", a=2),
                        pst4[:, :, 0, :], ucp[:])

            # ---------- phase 5: build S + counts ----------
            counts = pp.tile([128, 8], F32, tag="counts")
            s_t = [pp.tile([128, B], F16, tag=f"s{t}", name=f"s{t}")
                   for t in range(8)]
            for t in range(8):
                nc.vector.tensor_scalar(
                    out=s_t[t][:], in0=tpb[:], scalar1=io_sb[:, t:t + 1],
                    scalar2=None, op0=OP.is_equal, op1=OP.add,
                    accum_out=counts[:, t:t + 1])

            cm = pp.tile([128, 8], F32, tag="cm")
            cinv = pp.tile([128, 8], F32, tag="cinv")
            c16 = pp.tile([128, 8], F16, tag="c16")
            nc.vector.tensor_scalar_max(cm[:], counts[:], 1.0)
            nc.vector.reciprocal(out=cinv[:], in_=cm[:])
            nc.scalar.activation(out=c16[:], in_=counts[:], func=ACT.Copy)

            # ---------- phase 6: W^T in fp16 ----------
            wt16 = [pp.tile([128, RPC], F16, tag=f"wt{t}", name=f"wt{t}")
                    for t in range(8)]
            for t in range(8):
                nc.gpsimd.tensor_mul(ytb_t[t][:], ytb_t[t][:], psb_t[t][:])
                nc.scalar.activation(out=wt16[t][:], in_=ytb_t[t][:],
                                     func=ACT.Copy, scale=cinv[:, t:t + 1])

            # ---------- phase 8: mask matmul + masked row sums ----------
            s1 = pp.tile([128, 4], F32, tag="s1a")
            mdc = pp.tile([128, 4, 4], F32, tag="mdc")
            for m in range(4):
                psk = psp.tile([128, 2048], F32, tag="ps", name=f"psmask{m}")
                for t in range(8):
                    lhsT = wt16[t][:, m * 128:(m + 1) * 128]
                    for n in range(4):
                        nc.tensor.matmul(
                            psk[:, n * 512:(n + 1) * 512], lhsT,
                            s_t[t][:, n * 512:(n + 1) * 512],
                            start=(t == 0), stop=(t == 7))
                tsc = sp.tile([128, 2048], F32, tag="tscr")
                nc.vector.scalar_tensor_tensor(
                    out=tsc[:], in0=psk[:], scalar=1.0, in1=qraw[m][:],
                    op0=OP.mult, op1=OP.mult,
                    accum_out=s1[:, m:m + 1])
                for n in range(4):
                    t128 = sp.tile([128, 128], F32, tag="t128")
                    nc.vector.scalar_tensor_tensor(
                        out=t128[:],
                        in0=psk[:, n * 512 + m * 128:n * 512 + (m + 1) * 128],
                        scalar=1.0, in1=eye_sb[:],
                        op0=OP.mult, op1=OP.mult,
                        accum_out=mdc[:, m, n:n + 1])

            # ---------- phase 7: Mh via counts matmul ----------
            ps_m = psp.tile([128, 4], F32, tag="ps", name="ps_m")
            mh = pp.tile([128, 4], F32, tag="mh")
            for m in range(4):
                for t in range(8):
                    nc.tensor.matmul(
                        ps_m[:, m:m + 1],
                        wt16[t][:, m * 128:(m + 1) * 128],
                        c16[:, t:t + 1],
                        start=(t == 0), stop=(t == 7))
            nc.vector.tensor_copy(out=mh[:], in_=ps_m[:])

            # ---------- phase 9: per-row loss ----------
            zsum = pp.tile([128, 4], F32, tag="zsum")
            zz = pp.tile([128, 4], F32, tag="zz")
            lnz = pp.tile([128, 4], F32, tag="lnz")
            nc.vector.reduce_sum(zsum[:], zacc[:], axis=AX.X)
            nc.vector.tensor_scalar_add(zz[:], zsum[:], -1.0)
            nc.scalar.activation(out=lnz[:], in_=zz[:], func=ACT.Ln)

            md = pp.tile([128, 4], F32, tag="md")
            for m in range(4):
                t4 = sp.tile([128, 4], F32, tag="t4")
                nc.vector.scalar_tensor_tensor(
                    out=t4[:], in0=mdc[:, m, :], scalar=1.0, in1=dsel_sb[:],
                    op0=OP.mult, op1=OP.mult,
                    accum_out=md[:, m:m + 1])

            t1 = pp.tile([128, 4], F32, tag="t1")
            t2 = pp.tile([128, 4], F32, tag="t2")
            u1 = pp.tile([128, 4], F32, tag="u1")
            rl = pp.tile([128, 4], F32, tag="rl")
            rlp = pp.tile([128, 1], F32, tag="rlp")
            # t1 = 2*Mh - md ; t2 = lnz*t1 ; u1 = 10*s1 - t2 ; rl = -20*Mh + u1
            nc.vector.scalar_tensor_tensor(out=t1[:], in0=mh[:], scalar=2.0,
                                           in1=md[:], op0=OP.mult,
                                           op1=OP.subtract)
            nc.vector.tensor_mul(t2[:], lnz[:], t1[:])
            nc.vector.scalar_tensor_tensor(out=u1[:], in0=s1[:], scalar=10.0,
                                           in1=t2[:], op0=OP.mult,
                                           op1=OP.subtract)
            nc.vector.scalar_tensor_tensor(out=rl[:], in0=mh[:], scalar=-20.0,
                                           in1=u1[:], op0=OP.mult, op1=OP.add)
            nc.vector.reduce_sum(rlp[:], rl[:], axis=AX.X)
            nc.sync.dma_start(out=out_loss.ap(), in_=rlp[:])

            # ---------- phase 10: new_target ----------
            snt = pp.tile([128, 2], F32, tag="snt")
            rin = pp.tile([128, 2], F32, tag="rin")
            for j in range(2):
                mx = sp.tile([128, 1], F32, tag="ntmx")
                nmx = sp.tile([128, 1], F32, tag="ntnmx")
                nc.vector.reduce_max(mx[:], osh_t[j][:], axis=AX.X)
                nc.scalar.activation(out=nmx[:], in_=mx[:], func=ACT.Copy,
                                     scale=-1.0)
                ee = sp.tile([128, C], F32, tag="ntee")
                nc.scalar.activation(out=ee[:], in_=osh_t[j][:], func=ACT.Exp,
                                     bias=nmx[:], scale=1.0)
                tt = sp.tile([128, C], F32, tag="nttt")
                nc.vector.scalar_tensor_tensor(
                    out=tt[:], in0=ee[:], scalar=1.0, in1=ysh_t[j][:],
                    op0=OP.mult, op1=OP.mult,
                    accum_out=snt[:, j:j + 1])
                nc.vector.reciprocal(out=rin[:, j:j + 1], in_=snt[:, j:j + 1])
                nc.scalar.activation(out=tt[:], in_=tt[:], func=ACT.Copy,
                                     scale=rin[:, j:j + 1])
                nc.sync.dma_start(out=out_nt.ap()[j * 128:(j + 1) * 128, :],
                                  in_=tt[:])

    nc.compile()
    return nc


_NC_CACHE = None


def _get_graph():
    global _NC_CACHE
    if _NC_CACHE is None:
        _NC_CACHE = build_graph()
    return _NC_CACHE


def make_in_maps(outputs, features, Y, predicted_score):
    outputs = np.ascontiguousarray(outputs, dtype=np.float32)
    features = np.ascontiguousarray(features, dtype=np.float32)
    Y = np.ascontiguousarray(Y, dtype=np.float32)
    ps = np.ascontiguousarray(predicted_score, dtype=np.float32)

    cf = np.concatenate([features[:, 0, :], features[:, 1, :]], axis=0)
    cft = np.ascontiguousarray(cf.T.astype(np.float16))     # [256, 4096]
    in_maps = []
    for r in range(NCORES):
        ablk = 512 * (r % 4)
        ytb = np.zeros((CP, RPC), dtype=np.float32)
        psb = np.zeros((CP, RPC), dtype=np.float32)
        ytb[:C] = Y[ablk:ablk + RPC].T
        psb[:C] = ps[ablk:ablk + RPC].T
        dsel = np.zeros((128, 4), dtype=np.float32)
        dsel[:, r % 4] = 1.0
        in_maps.append({
            "cfl": np.ascontiguousarray(cft[:, r * RPC:(r + 1) * RPC]),
            "cft": cft,
            "osh": np.ascontiguousarray(outputs[r * BSH:(r + 1) * BSH]),
            "ysh": np.ascontiguousarray(Y[r * BSH:(r + 1) * BSH]),
            "ytb": ytb,
            "psb": psb,
            "dsel": dsel,
        })
    return in_maps


def assemble(results):
    new_target = np.concatenate(
        [results[r]["out_nt"] for r in range(NCORES)], axis=0)
    total = np.float64(0.0)
    for r in range(NCORES):
        total += np.float64(results[r]["out_loss"].sum())
    loss = -(10.0 / 7.0) * total / float(TWOB)
    return np.float32(loss), new_target.astype(np.float32)


def kernel(outputs, features, Y, predicted_score):
    nc = _get_graph()
    in_maps = make_in_maps(outputs, features, Y, predicted_score)
    res = run_bass_kernel_spmd(nc, in_maps, core_ids=list(range(NCORES)))
    return assemble(res.results)


# revision 17
# speedup vs baseline: 2.6869x; 1.2323x over previous
"""Distributed Trainium2 Bass kernel for the SupCon-style partial-label
contrastive loss (nn_ABLE_48198122995803).

Sharding: the 4096 anchor rows of the logits matrix are split across the 8
NeuronCores (512 rows each).  contrast_feature^T is replicated to every core
(in fp16); each core computes its row-block of logits, the masked row
reductions, and a partial loss sum.  The pseudo-label argmax (tp) is sharded
8x256 rows and exchanged with one tiny fp16 AllGather.  new_target is
row-sharded 8x256.

Math notes (vs reference.py):
 - features are L2-normalized, so row-max(adc) == adc_ii ~= 10.  log_prob is
   invariant to the per-row subtracted constant, so we use the constant 10;
   exp(l_ii) ~= 1 (error < 2e-5) is excluded from the denominator as 1.0.
 - mask[a,b] = W[a, tp[b]] is the one-hot matmul W^T.T @ S with
   S[c,b] = (tp[b]==c) in fp16 (exact gather; W rounded to fp16, err 5e-4).
 - per-row: row_loss = 10*s1 - 20*Mh - lnZ*(2*Mh - md), where
   s1 = sum_b mask[a,b]*qraw[i,b], qraw = dot[i,b]+dot[i,b+2048],
   Mh = sum_b mask[a,b] (counts matmul), md = mask[a,a].
   loss = -(10/7) * mean(row_loss).

Layout: adc PSUM tiles are [128, 2048] (4 banks), two per row-tile m in
chunk-pair order (n0, n0+4, n1, n1+4) so the qraw pair-adds read one tile;
one big ScalarE exp+accum per tile gives the Z row-sums.
"""

import numpy as np

import concourse.bass as bass
import concourse.tile as tile
from concourse import bacc, mybir
from concourse.bass_utils import run_bass_kernel_spmd

B = 2048
C = 1000
CP = 1024          # class dim padded to 8*128
D = 256
TWOB = 4096
NCORES = 8
RPC = TWOB // NCORES   # 512 logits rows per core
BSH = B // NCORES      # 256 softmax/new_target rows per core

F32 = mybir.dt.float32
F16 = mybir.dt.float16
U16 = mybir.dt.uint16
AX = mybir.AxisListType
OP = mybir.AluOpType
ACT = mybir.ActivationFunctionType


def build_graph():
    nc = bacc.Bacc("TRN2", target_bir_lowering=False, debug=False,
                   num_devices=NCORES)

    # ---- per-core external I/O ------------------------------------------
    cfl = nc.dram_tensor("cfl", [D, RPC], F16, kind="ExternalInput")
    cft = nc.dram_tensor("cft", [D, TWOB], F16, kind="ExternalInput")
    osh = nc.dram_tensor("osh", [BSH, C], F32, kind="ExternalInput")
    ysh = nc.dram_tensor("ysh", [BSH, C], F32, kind="ExternalInput")
    ytb = nc.dram_tensor("ytb", [CP, RPC], F32, kind="ExternalInput")
    psb = nc.dram_tensor("psb", [CP, RPC], F32, kind="ExternalInput")
    dsel = nc.dram_tensor("dsel", [128, 4], F32, kind="ExternalInput")
    out_nt = nc.dram_tensor("out_nt", [BSH, C], F32, kind="ExternalOutput")
    out_loss = nc.dram_tensor("out_loss", [128, 1], F32, kind="ExternalOutput")

    # ---- inline constants ----------------------------------------------
    eye_np = np.eye(128, dtype=np.float32)
    io_np = (np.arange(128, dtype=np.float32)[:, None]
             + 128.0 * np.arange(8, dtype=np.float32)[None, :])
    eye_dr = nc.inline_tensor(eye_np, name="eye_c")
    io_dr = nc.inline_tensor(io_np, name="io_c")
    one_dr = nc.inline_tensor(np.ones((1, 128), dtype=np.float16), name="one_c")

    # ---- collective bounce buffers -------------------------------------
    cc_in = nc.dram_tensor("cc_in", [BSH], F16, kind="Internal",
                           addr_space="Local")
    cc_out = nc.dram_tensor("cc_out", [B], F16, kind="Internal",
                            addr_space="Shared")

    with tile.TileContext(nc) as tc:
        with (
            tc.tile_pool(name="persist", bufs=1) as pp,
            tc.tile_pool(name="scr", bufs=2) as sp,
            tc.tile_pool(name="psum", bufs=2, space="PSUM") as psp,
        ):
            # ---------- constants into SBUF ----------
            eye_sb = pp.tile([128, 128], F32, tag="eye")
            io_sb = pp.tile([128, 8], F32, tag="io")
            one_sb = pp.tile([1, 128], F16, tag="one")
            dsel_sb = pp.tile([128, 4], F32, tag="dsel")
            nb10 = pp.tile([128, 1], F32, tag="nb10")
            nc.vector.memset(nb10[:], -10.0)
            nc.sync.dma_start(out=eye_sb[:], in_=eye_dr.ap())
            nc.sync.dma_start(out=io_sb[:], in_=io_dr.ap())
            nc.sync.dma_start(out=one_sb[:], in_=one_dr.ap())
            nc.sync.dma_start(out=dsel_sb[:], in_=dsel.ap())

            # ---------- input DMAs: cf first (feeds TensorE), queues spread
            cfl_t, cft_t = [], []
            for k in range(2):
                lt = pp.tile([128, RPC], F16, tag=f"cfl{k}")
                ft = pp.tile([128, TWOB], F16, tag=f"cft{k}")
                nc.sync.dma_start(out=lt[:], in_=cfl.ap()[k * 128:(k + 1) * 128, :])
                nc.sync.dma_start(out=ft[:], in_=cft.ap()[k * 128:(k + 1) * 128, :])
                cfl_t.append(lt)
                cft_t.append(ft)
            osh_t, ysh_t = [], []
            for j in range(2):
                ot = pp.tile([128, C], F32, tag=f"osh{j}")
                yt = pp.tile([128, C], F32, tag=f"ysh{j}")
                nc.scalar.dma_start(out=ot[:], in_=osh.ap()[j * 128:(j + 1) * 128, :])
                nc.scalar.dma_start(out=yt[:], in_=ysh.ap()[j * 128:(j + 1) * 128, :])
                osh_t.append(ot)
                ysh_t.append(yt)
            ytb_t, psb_t = [], []
            for t in range(8):
                yt = pp.tile([128, RPC], F32, tag=f"ytb{t}")
                pt = pp.tile([128, RPC], F32, tag=f"psb{t}")
                nc.gpsimd.dma_start(out=yt[:], in_=ytb.ap()[t * 128:(t + 1) * 128, :])
                nc.gpsimd.dma_start(out=pt[:], in_=psb.ap()[t * 128:(t + 1) * 128, :])
                ytb_t.append(yt)
                psb_t.append(pt)
            # W^T products early on GpSimd (independent of the collective)
            for t in range(8):
                nc.gpsimd.tensor_mul(ytb_t[t][:], ytb_t[t][:], psb_t[t][:])

            # ---------- phase 1: tp shard (argmax over candidates) ----------
            tp16 = pp.tile([128, 2], F16, tag="tp16")
            for j in range(2):
                v = sp.tile([128, C], F32, tag="vmask")
                # v = (o + 1000) * Y : >0 on candidates, 0 elsewhere
                nc.vector.scalar_tensor_tensor(
                    out=v[:], in0=osh_t[j][:], scalar=1000.0, in1=ysh_t[j][:],
                    op0=OP.add, op1=OP.mult)
                mx8 = sp.tile([128, 8], F32, tag="mx8")
                idx8 = sp.tile([128, 8], U16, tag="idx8")
                nc.vector.max(out=mx8[:], in_=v[:])
                nc.vector.max_index(out=idx8[:], in_max=mx8[:], in_values=v[:])
                idf = sp.tile([128, 1], F32, tag="idf")
                nc.vector.tensor_copy(out=idf[:], in_=idx8[:, 0:1])
                nc.vector.tensor_copy(out=tp16[:, j:j + 1], in_=idf[:])

            # ---------- phase 2: AllGather tp ----------
            for j in range(2):
                nc.gpsimd.dma_start(out=cc_in.ap()[j * 128:(j + 1) * 128],
                                    in_=tp16[:, j:j + 1])
            nc.gpsimd.collective_compute(
                "AllGather", OP.bypass,
                replica_groups=[list(range(NCORES))],
                ins=[cc_in.ap().opt()],
                outs=[cc_out.ap().opt()],
            )
            tpb = pp.tile([128, B], F16, tag="tpb")
            cc_bcast = bass.AP(tensor=cc_out.ap().tensor, offset=0,
                               ap=[[0, 128], [1, B]])
            nc.gpsimd.dma_start(out=tpb[:], in_=cc_bcast)

            # ---------- phase 4: adc matmul, exp/Z, qraw ----------
            # psum tile per (m, half): columns = chunks (n0, n0+4, n1, n1+4),
            # n0 = 2h, n1 = 2h+1.  S-build ops are interleaved into the DVE
            # stream so they fill DVE idle slots without head-of-line
            # blocking the psum-consuming qraw adds.
            counts = pp.tile([128, 8], F32, tag="counts")
            s_t = [pp.tile([128, B], F16, tag=f"s{t}", name=f"s{t}")
                   for t in range(8)]
            zacc = pp.tile([128, 4, 2], F32, tag="zacc")
            qraw = [pp.tile([128, B], F16, tag=f"qraw{m}", name=f"qraw{m}")
                    for m in range(4)]
            for m in range(4):
                for h in range(2):
                    pst = psp.tile([128, 2048], F32, tag="ps",
                                   name=f"psadc{m}_{h}")
                    chunk_cols = [2 * h, 2 * h + 4, 2 * h + 1, 2 * h + 5]
                    for k in range(2):
                        lhsT = cfl_t[k][:, m * 128:(m + 1) * 128]
                        for c, n in enumerate(chunk_cols):
                            nc.tensor.matmul(
                                pst[:, c * 512:(c + 1) * 512],
                                lhsT,
                                cft_t[k][:, n * 512:(n + 1) * 512],
                                start=(k == 0), stop=(k == 1))
                    # Z partial: one exp+accum over the whole 4-bank tile
                    esc = sp.tile([128, 2048], F32, tag="escr")
                    nc.scalar.activation(
                        out=esc[:], in_=pst[:], func=ACT.Exp,
                        bias=nb10[:], scale=10.0,
                        accum_out=zacc[:, m, h:h + 1])
                    # qraw pair sums: pst as [128, 2 pairs, 2 half, 512]
                    pst4 = pst[:].rearrange("p (a b c) -> p a b c", a=2, b=2)
                    ucp = sp.tile([128, 2, 512], F32, tag="ucp",
                                  name=f"ucp{m}_{h}")
                    nc.scalar.activation(out=ucp[:], in_=pst4[:, :, 1, :],
                                         func=ACT.Copy)
                    nc.vector.tensor_add(
                        qraw[m][:, h * 1024:(h + 1) * 1024]
                        .rearrange("p (a c) -> p a c", a=2),
                        pst4[:, :, 0, :], ucp[:])
                for t in (2 * m, 2 * m + 1):
                    nc.vector.tensor_scalar(
                        out=s_t[t][:], in0=tpb[:], scalar1=io_sb[:, t:t + 1],
                        scalar2=None, op0=OP.is_equal, op1=OP.add,
                        accum_out=counts[:, t:t + 1])

            cm = pp.tile([128, 8], F32, tag="cm")
            cinv = pp.tile([128, 8], F32, tag="cinv")
            c16 = pp.tile([128, 8], F16, tag="c16")
            nc.vector.tensor_scalar_max(cm[:], counts[:], 1.0)
            nc.vector.reciprocal(out=cinv[:], in_=cm[:])
            nc.scalar.activation(out=c16[:], in_=counts[:], func=ACT.Copy)

            # ---------- phase 6: W^T in fp16 (scale + cast) ----------
            wt16 = [pp.tile([128, RPC], F16, tag=f"wt{t}", name=f"wt{t}")
                    for t in range(8)]
            for t in range(8):
                nc.scalar.activation(out=wt16[t][:], in_=ytb_t[t][:],
                                     func=ACT.Copy, scale=cinv[:, t:t + 1])

            # ---------- phase 8: mask matmul + masked row sums ----------
            s1 = pp.tile([128, 4], F32, tag="s1a")
            mdc = pp.tile([128, 4, 4], F32, tag="mdc")
            for m in range(4):
                psk = psp.tile([128, 2048], F32, tag="ps", name=f"psmask{m}")
                for t in range(8):
                    lhsT = wt16[t][:, m * 128:(m + 1) * 128]
                    for n in range(4):
                        nc.tensor.matmul(
                            psk[:, n * 512:(n + 1) * 512], lhsT,
                            s_t[t][:, n * 512:(n + 1) * 512],
                            start=(t == 0), stop=(t == 7))
                tsc = sp.tile([128, 2048], F32, tag="tscr")
                nc.vector.scalar_tensor_tensor(
                    out=tsc[:], in0=psk[:], scalar=1.0, in1=qraw[m][:],
                    op0=OP.mult, op1=OP.mult,
                    accum_out=s1[:, m:m + 1])
                for n in range(4):
                    t128 = sp.tile([128, 128], F32, tag="t128")
                    nc.vector.scalar_tensor_tensor(
                        out=t128[:],
                        in0=psk[:, n * 512 + m * 128:n * 512 + (m + 1) * 128],
                        scalar=1.0, in1=eye_sb[:],
                        op0=OP.mult, op1=OP.mult,
                        accum_out=mdc[:, m, n:n + 1])

            # ---------- phase 7: Mh via counts matmul ----------
            ps_m = psp.tile([128, 4], F32, tag="ps", name="ps_m")
            mh = pp.tile([128, 4], F32, tag="mh")
            for m in range(4):
                for t in range(8):
                    nc.tensor.matmul(
                        ps_m[:, m:m + 1],
                        wt16[t][:, m * 128:(m + 1) * 128],
                        c16[:, t:t + 1],
                        start=(t == 0), stop=(t == 7))
            nc.vector.tensor_copy(out=mh[:], in_=ps_m[:])

            # ---------- phase 9: per-row loss ----------
            zsum = pp.tile([128, 4], F32, tag="zsum")
            zz = pp.tile([128, 4], F32, tag="zz")
            lnz = pp.tile([128, 4], F32, tag="lnz")
            nc.vector.reduce_sum(zsum[:], zacc[:], axis=AX.X)
            nc.vector.tensor_scalar_add(zz[:], zsum[:], -1.0)
            nc.scalar.activation(out=lnz[:], in_=zz[:], func=ACT.Ln)

            md = pp.tile([128, 4], F32, tag="md")
            for m in range(4):
                t4 = sp.tile([128, 4], F32, tag="t4")
                nc.vector.scalar_tensor_tensor(
                    out=t4[:], in0=mdc[:, m, :], scalar=1.0, in1=dsel_sb[:],
                    op0=OP.mult, op1=OP.mult,
                    accum_out=md[:, m:m + 1])

            t1 = pp.tile([128, 4], F32, tag="t1")
            t2 = pp.tile([128, 4], F32, tag="t2")
            u1 = pp.tile([128, 4], F32, tag="u1")
            rl = pp.tile([128, 4], F32, tag="rl")
            rlp = pp.tile([128, 1], F32, tag="rlp")
            # t1 = 2*Mh - md ; t2 = lnz*t1 ; u1 = 10*s1 - t2 ; rl = -20*Mh + u1
            nc.vector.scalar_tensor_tensor(out=t1[:], in0=mh[:], scalar=2.0,
                                           in1=md[:], op0=OP.mult,
                                           op1=OP.subtract)
            nc.vector.tensor_mul(t2[:], lnz[:], t1[:])
            nc.vector.scalar_tensor_tensor(out=u1[:], in0=s1[:], scalar=10.0,
                                           in1=t2[:], op0=OP.mult,
                                           op1=OP.subtract)
            nc.vector.scalar_tensor_tensor(out=rl[:], in0=mh[:], scalar=-20.0,
                                           in1=u1[:], op0=OP.mult, op1=OP.add)
            nc.vector.reduce_sum(rlp[:], rl[:], axis=AX.X)
            nc.sync.dma_start(out=out_loss.ap(), in_=rlp[:])

            # ---------- phase 10: new_target ----------
            snt = pp.tile([128, 2], F32, tag="snt")
            rin = pp.tile([128, 2], F32, tag="rin")
            for j in range(2):
                mx = sp.tile([128, 1], F32, tag="ntmx")
                nmx = sp.tile([128, 1], F32, tag="ntnmx")
                nc.vector.reduce_max(mx[:], osh_t[j][:], axis=AX.X)
                nc.scalar.activation(out=nmx[:], in_=mx[:], func=ACT.Copy,
                                     scale=-1.0)
                ee = sp.tile([128, C], F32, tag="ntee")
                nc.scalar.activation(out=ee[:], in_=osh_t[j][:], func=ACT.Exp,
                                     bias=nmx[:], scale=1.0)
                tt = sp.tile([128, C], F32, tag="nttt")
                nc.vector.scalar_tensor_tensor(
                    out=tt[:], in0=ee[:], scalar=1.0, in1=ysh_t[j][:],
                    op0=OP.mult, op1=OP.mult,
                    accum_out=snt[:, j:j + 1])
                nc.vector.reciprocal(out=rin[:, j:j + 1], in_=snt[:, j:j + 1])
                nc.scalar.activation(out=tt[:], in_=tt[:], func=ACT.Copy,
                                     scale=rin[:, j:j + 1])
                nc.sync.dma_start(out=out_nt.ap()[j * 128:(j + 1) * 128, :],
                                  in_=tt[:])

    nc.compile()
    return nc


_NC_CACHE = None


def _get_graph():
    global _NC_CACHE
    if _NC_CACHE is None:
        _NC_CACHE = build_graph()
    return _NC_CACHE


def make_in_maps(outputs, features, Y, predicted_score):
    outputs = np.ascontiguousarray(outputs, dtype=np.float32)
    features = np.ascontiguousarray(features, dtype=np.float32)
    Y = np.ascontiguousarray(Y, dtype=np.float32)
    ps = np.ascontiguousarray(predicted_score, dtype=np.float32)

    cf = np.concatenate([features[:, 0, :], features[:, 1, :]], axis=0)
    cft = np.ascontiguousarray(cf.T.astype(np.float16))     # [256, 4096]
    in_maps = []
    for r in range(NCORES):
        ablk = 512 * (r % 4)
        ytb = np.zeros((CP, RPC), dtype=np.float32)
        psb = np.zeros((CP, RPC), dtype=np.float32)
        ytb[:C] = Y[ablk:ablk + RPC].T
        psb[:C] = ps[ablk:ablk + RPC].T
        dsel = np.zeros((128, 4), dtype=np.float32)
        dsel[:, r % 4] = 1.0
        in_maps.append({
            "cfl": np.ascontiguousarray(cft[:, r * RPC:(r + 1) * RPC]),
            "cft": cft,
            "osh": np.ascontiguousarray(outputs[r * BSH:(r + 1) * BSH]),
            "ysh": np.ascontiguousarray(Y[r * BSH:(r + 1) * BSH]),
            "ytb": ytb,
            "psb": psb,
            "dsel": dsel,
        })
    return in_maps


def assemble(results):
    new_target = np.concatenate(
        [results[r]["out_nt"] for r in range(NCORES)], axis=0)
    total = np.float64(0.0)
    for r in range(NCORES):
        total += np.float64(results[r]["out_loss"].sum())
    loss = -(10.0 / 7.0) * total / float(TWOB)
    return np.float32(loss), new_target.astype(np.float32)


def kernel(outputs, features, Y, predicted_score):
    nc = _get_graph()
    in_maps = make_in_maps(outputs, features, Y, predicted_score)
    res = run_bass_kernel_spmd(nc, in_maps, core_ids=list(range(NCORES)))
    return assemble(res.results)
